# revision 7
# baseline (speedup 1.0000x reference)
"""BiCGSTAB solver for nn_BiCG_Net on 8 TRN2 NeuronCores (pure data parallel).

v2: each core solves one (b,c) 384x384 5-point stencil system with KMAX=30
BiCGSTAB iterations, fully SBUF/PSUM-resident.

Layout: grid row i lives at (partition p = i//3, row r = i%3), free index
f = 384*r + j. to_dev is a plain reshape(128, 1152).

apply_A strategy: host pre-shifts the four neighbor coefficient tensors so
the five elementwise products are offset-free; the shifts AND the 5-term sum
are then done by 15 cheap float32r PE matmuls (identity / partition-shift
matrices) accumulating into PSUM. Edge (symmetric-pad) contributions are
folded into the center coefficient on the host, so no edge fixup ops exist
on device.
"""

import numpy as np

import concourse.bass as bass
import concourse.bacc as bacc
import concourse.mybir as mybir
import concourse.tile as tile
from concourse import bass_utils

F32 = np.float32
N = 384
P = 128
RPB = 3            # grid rows per partition
W = RPB * N        # 1152
KMAX = 30
EPS = 1e-9
THR2 = float(F32(EPS * N * N)) ** 2     # squared-norm threshold
EPS2 = float(F32(EPS)) ** 2

ET = mybir.EngineType

# ---------------- scalar slots in SC[128, NSLOT] ----------------
# first 8 loaded straight from host scal[P, 8]
(RHO, RABS2, R0SQ, KEPS, CC, CONV, NOTCONV, SGC,
 RESNEG, SS2, VVK, RESP, RES, FR1, FNR, NOTFNR, SIGS, RECA, ALPHA,
 ALPHAX, NALPHAX, C2, NOTC2, FC4, FC3, NOTFC4, TTS, RECB, OMEGA,
 OMEGAX, NOMEGAX, OMS, RECC, DENS, RECD, Q1, Q2, BETA, BETAX,
 FPFIX) = range(40)
NSLOT = 40


# ======================= host-side precompute =======================

def _sym_pad2(a):
    return np.pad(a, ((1, 1), (1, 1)), mode='symmetric')


def host_prepare(V, mask1, mask2):
    """Stencil coeffs in the transposed working frame, edge-folded and
    pre-shifted for the v2 device program. All f32, matching reference op
    order."""
    Vt = np.ascontiguousarray(V.T)
    m1 = np.ascontiguousarray(mask1.T)
    m2 = np.ascontiguousarray(mask2.T)
    Vp = (_sym_pad2(Vt) + F32(1.0)).astype(F32)
    m1p = _sym_pad2(m1).astype(F32)
    m2p = _sym_pad2(m2).astype(F32)

    d1r = ((Vp[1:, :] - Vp[:-1, :]) / (F32(0.5) * (Vp[1:, :] + Vp[:-1, :]))).astype(F32)
    d2r = ((Vp[:, 1:] - Vp[:, :-1]) / (F32(0.5) * (Vp[:, 1:] + Vp[:, :-1]))).astype(F32)
    d1 = np.zeros((N + 2, N + 2), F32)
    d1[:N + 1, 1:N + 1] = d1r[:, 1:N + 1]
    d1 = (d1 * m1p).astype(F32)
    d2 = np.zeros((N + 2, N + 2), F32)
    d2[1:N + 1, :N + 1] = d2r[1:N + 1, :]
    d2 = (d2 * m2p).astype(F32)
    rx = F32(5.0)
    rxx = F32(10.0)
    dd1 = (np.pad(d1, ((1, 0), (0, 0)))[:-1, :] - d1).astype(F32)
    dd2 = (np.pad(d2, ((0, 0), (1, 0)))[:, :-1] - d2).astype(F32)
    boo = (F32(1.0) + F32(2.0) * (rxx + rxx) - rx * dd1 - rx * dd2)[1:N + 1, 1:N + 1].astype(F32)
    bpo = (-rxx + rx * d1[1:N + 1, 1:N + 1]).astype(F32)
    bop = (-rxx + rx * d2[1:N + 1, 1:N + 1]).astype(F32)
    bmo = (-rxx - rx * d1[:N, 1:N + 1]).astype(F32)
    bom = (-rxx - rx * d2[1:N + 1, :N]).astype(F32)

    # edge folding (symmetric pad: out-of-range neighbor == center)
    cC = boo.copy()
    cC[0, :] += bmo[0, :]
    cC[N - 1, :] += bpo[N - 1, :]
    cC[:, 0] += bom[:, 0]
    cC[:, N - 1] += bop[:, N - 1]
    # pre-shifted neighbor coefficients: q*[idx] = c*[idx] * z[idx], then
    # out[i,j] += qU[i-1,j] / qD[i+1,j] / qL[f-1] / qR[f+1]
    cU = np.zeros((N, N), F32); cU[:N - 1, :] = bmo[1:, :]
    cD = np.zeros((N, N), F32); cD[1:, :] = bpo[:N - 1, :]
    cL = np.zeros((N, N), F32); cL[:, :N - 1] = bom[:, 1:]
    cR = np.zeros((N, N), F32); cR[:, 1:] = bop[:, :N - 1]

    c = F32(np.mean(V, dtype=F32) + F32(1.0))
    ax0 = ((((boo * c + bmo * c) + bom * c) + bop * c) + bpo * c).astype(F32)
    p0 = (c - ax0).astype(F32)
    x0 = np.full((N, N), c, F32)
    rho0 = F32(np.sum((p0 * p0).astype(F32), dtype=F32))
    return dict(cC=cC, cU=cU, cD=cD, cL=cL, cR=cR,
                x0=x0, p0=p0, c=c, rho0=rho0)


def to_dev(a):
    return np.ascontiguousarray(a.reshape(P, W))


def make_mats():
    I = np.eye(P, dtype=F32)
    Su = np.eye(P, k=1).astype(F32)   # out[m] = in[m-1], out[0] = 0
    Sd = np.eye(P, k=-1).astype(F32)  # out[m] = in[m+1], out[127] = 0
    return np.stack([I, Su, Sd])


# ======================= device program =======================

def build_nc(kmax=KMAX):
    nc = bacc.Bacc("TRN2", debug=False, num_devices=8)
    dt = mybir.dt.float32
    f32r = mybir.dt.float32r
    u32 = mybir.dt.uint32
    OP = mybir.AluOpType
    AF = mybir.ActivationFunctionType

    din = {}
    for nm in ("cC", "cU", "cD", "cL", "cR", "x0", "p0"):
        din[nm] = nc.dram_tensor(nm, [P, W], dt, kind="ExternalInput").ap()
    scal_in = nc.dram_tensor("scal", [P, 8], dt, kind="ExternalInput").ap()
    mats_in = nc.dram_tensor("mats", [3, P, P], f32r, kind="ExternalInput").ap()
    ones_in = nc.dram_tensor("ones", [P, P], dt, kind="ExternalInput").ap()
    zcol_in = nc.dram_tensor("zcol", [P, 1], f32r, kind="ExternalInput").ap()
    xout = nc.dram_tensor("xout", [P, W], dt, kind="ExternalOutput").ap()

    with tile.TileContext(nc) as tc:
        import contextlib
        with contextlib.ExitStack() as ctx:
            big = ctx.enter_context(tc.tile_pool(name="big", bufs=1))
            small = ctx.enter_context(tc.tile_pool(name="small", bufs=1))
            psum = ctx.enter_context(tc.tile_pool(name="psum", bufs=1, space="PSUM"))

            cf = {nm: big.tile([P, W], dt, tag=nm, name=nm) for nm in
                  ("cC", "cU", "cD", "cL", "cR")}
            x = big.tile([P, W], dt, tag="x")
            r = big.tile([P, W], dt, tag="r")
            r0 = big.tile([P, W], dt, tag="r0")
            pA = big.tile([P, W], dt, tag="pA")
            pB = big.tile([P, W], dt, tag="pB")
            s = big.tile([P, W], dt, tag="s")
            u = big.tile([P, W], dt, tag="u")
            TSa = big.tile([P, W], dt, tag="TSa")
            TSb = big.tile([P, W], dt, tag="TSb")
            TSc = big.tile([P, W], dt, tag="TSc")
            TSd = big.tile([P, W], dt, tag="TSd")
            # two q-sets (A(p) and A(s))
            qs = []
            for b in range(2):
                qc = big.tile([P, W], f32r, tag=f"qc{b}")
                qu = big.tile([P, W], f32r, tag=f"qu{b}")
                qd = big.tile([P, W], f32r, tag=f"qd{b}")
                qL = big.tile([P, W + 1], f32r, tag=f"qL{b}")
                qR = big.tile([P, W + 1], f32r, tag=f"qR{b}")
                qs.append((qc, qu, qd, qL, qR))

            SC = small.tile([P, NSLOT], dt, tag="SC")
            PT = small.tile([P, 8], dt, tag="PT")
            I_ = small.tile([P, P], f32r, tag="I_")
            Su_ = small.tile([P, P], f32r, tag="Su_")
            Sd_ = small.tile([P, P], f32r, tag="Sd_")
            ones = small.tile([P, P], dt, tag="ones")

            vps = psum.tile([P, 3 * 512], dt, tag="vps")
            tps = psum.tile([P, 3 * 512], dt, tag="tps")
            ps_dots = psum.tile([P, 8], dt, tag="ps_dots")

            def S(k):
                return SC[:, k:k + 1]

            def r3(t):          # [P, W] SBUF tile -> [P, 3, 384] view
                return t[:].rearrange("p (g w) -> p g w", g=RPB)

            def p3(t):          # [P, 1536] PSUM tile -> [P, 3, 384] view
                return t[:].rearrange("p (g w) -> p g w", g=RPB)[:, :, 0:N]

            v3 = p3(vps)
            t3 = p3(tps)

            def ts_(out, in0, s1, s2, op0, op1=None):
                kw = {} if op1 is None else {"op1": op1}
                nc.vector.tensor_scalar(out=out, in0=in0, scalar1=s1,
                                        scalar2=s2, op0=op0, **kw)

            def tt_(out, in0, in1, op):
                nc.vector.tensor_tensor(out=out, in0=in0, in1=in1, op=op)

            def products(z, qset):
                qc, qu, qd, qL, qR = qset
                # DVE: qc, qd, qL ; GpSimd: qR, qu
                nc.vector.tensor_mul(qc[:, :], cf["cC"][:, :], z[:, :])
                nc.gpsimd.tensor_mul(qR[:, 0:W], cf["cR"][:, :], z[:, :])
                nc.vector.tensor_mul(qd[:, :], cf["cD"][:, :], z[:, :])
                nc.gpsimd.tensor_mul(qu[:, :], cf["cU"][:, :], z[:, :])
                nc.vector.tensor_mul(qL[:, 1:W + 1], cf["cL"][:, :], z[:, :])

            def stencil_mm(qset, outp):
                qc, qu, qd, qL, qR = qset
                o3 = outp[:].rearrange("p (g w) -> p g w", g=RPB)

                def mm(k, lhs, rhs_ap, start, stop):
                    nc.tensor.matmul(o3[:, k, 0:N], lhs[:, :], rhs_ap,
                                     start=start, stop=stop)
                for k in range(RPB):
                    ck = k * N
                    mm(k, I_, qc[:, ck:ck + N], True, False)
                    mm(k, I_, qR[:, ck + 1:ck + N + 1], False, False)
                    if k < RPB - 1:
                        mm(k, I_, qd[:, ck + N:ck + 2 * N], False, False)
                    else:
                        mm(k, Sd_, qd[:, 0:N], False, False)
                    mm(k, I_, qL[:, ck:ck + N], False, False)
                    if k == 0:
                        mm(k, Su_, qu[:, 2 * N:3 * N], False, True)
                    else:
                        mm(k, I_, qu[:, ck - N:ck], False, True)

            # ---------------- loads / prologue ----------------
            nc.sync.dma_start(SC[:, 0:8], scal_in)
            for nm in ("cC", "cU", "cD", "cL", "cR"):
                nc.sync.dma_start(cf[nm][:, :], din[nm])
            nc.sync.dma_start(x[:, :], din["x0"])
            nc.sync.dma_start(pA[:, :], din["p0"])
            for i, t in enumerate((I_, Su_, Sd_)):
                nc.sync.dma_start(t[:, :], mats_in[i])
            nc.sync.dma_start(ones[:, :], ones_in)
            nc.vector.tensor_copy(r[:, :], pA[:, :])
            nc.scalar.copy(r0[:, :], pA[:, :])
            for b in range(2):
                nc.sync.dma_start(qs[b][3][:, 0:1], zcol_in)      # qL guard
                nc.sync.dma_start(qs[b][4][:, W:W + 1], zcol_in)  # qR guard

            regs_r1 = nc.alloc_registers(
                "fr1", bass.OrderedSet([ET.DVE, ET.Pool, ET.Activation, ET.PE]))
            regs_fix = nc.alloc_registers("ffix", bass.OrderedSet([ET.DVE]))

            pcur, pnxt = pA, pB
            for it in range(kmax):
                # ---------- v = A(p) ----------
                products(pcur, qs[0])
                stencil_mm(qs[0], vps)
                # ---------- sigma = <v,r0>, vv = <v,v> ----------
                nc.vector.scalar_tensor_tensor(
                    out=r3(TSa), in0=v3, scalar=1.0, in1=r3(r0),
                    op0=OP.mult, op1=OP.mult, accum_out=PT[:, 0:1])
                nc.scalar.activation(r3(TSb), v3, AF.Square,
                                     accum_out=PT[:, 1:2])
                nc.tensor.matmul(ps_dots[:, 0:2], ones[:, :], PT[:, 0:2],
                                 start=True, stop=True)
                # ---------- RES flags (all DVE; sigma<=eps*|v||r0| via
                #            sigma<=0 or sigma^2 <= vv*KEPS) ----------
                ts_(S(RESNEG), ps_dots[:, 0:1], 0.0, None, OP.is_le)
                nc.vector.tensor_copy(S(SGC), ps_dots[:, 0:1])
                tt_(S(SS2), ps_dots[:, 0:1], S(SGC), OP.mult)
                tt_(S(VVK), ps_dots[:, 1:2], S(KEPS), OP.mult)
                tt_(S(RESP), S(SS2), S(VVK), OP.is_le)
                tt_(S(RES), S(RESNEG), S(RESP), OP.max)
                tt_(S(FR1), S(CONV), S(RES), OP.mult)
                tt_(S(FNR), S(CONV), S(FR1), OP.subtract)
                # ---------- restart branch (rare) ----------
                for reg in regs_r1:
                    nc.reg_load(reg, SC[0:1, FR1:FR1 + 1].bitcast(u32))
                with tc.If(nc.snap(regs_r1, donate=True) > 0):
                    products(x, qs[1])
                    stencil_mm(qs[1], tps)
                    nc.vector.tensor_scalar(out=r3(r), in0=t3, scalar1=-1.0,
                                            scalar2=S(CC), op0=OP.mult,
                                            op1=OP.add)
                    nc.scalar.copy(r0[:, :], r[:, :])
                    nc.scalar.activation(TSb[:, :], r[:, :], AF.Square,
                                         accum_out=PT[:, 7:8])
                    nc.tensor.matmul(ps_dots[:, 7:8], ones[:, :], PT[:, 7:8],
                                     start=True, stop=True)
                    nc.vector.tensor_copy(S(RHO), ps_dots[:, 7:8])
                    nc.vector.tensor_copy(S(RABS2), ps_dots[:, 7:8])
                    nc.vector.tensor_copy(S(R0SQ), ps_dots[:, 7:8])
                    ts_(S(KEPS), ps_dots[:, 7:8], EPS2, None, OP.mult)
                # ---------- alpha ----------
                ts_(S(NOTFNR), S(FNR), -1.0, 1.0, OP.mult, OP.add)
                ts_(S(SIGS), ps_dots[:, 0:1], S(FNR), S(NOTFNR),
                    OP.mult, OP.add)
                nc.vector.reciprocal(S(RECA), S(SIGS))
                ts_(S(ALPHA), S(RECA), S(RHO), None, OP.mult)
                ts_(S(ALPHAX), S(ALPHA), S(FNR), None, OP.mult)
                ts_(S(NALPHAX), S(ALPHAX), -1.0, None, OP.mult)
                # ---------- s = r - alphax*v ----------
                nc.vector.scalar_tensor_tensor(
                    out=r3(s), in0=v3, scalar=S(NALPHAX), in1=r3(r),
                    op0=OP.mult, op1=OP.add)
                # ---------- t = A(s) ----------
                products(s, qs[1])
                stencil_mm(qs[1], tps)
                # ---------- ss (off critical path) ----------
                nc.scalar.activation(TSb[:, :], s[:, :], AF.Square,
                                     accum_out=PT[:, 2:3])
                nc.tensor.matmul(ps_dots[:, 2:3], ones[:, :], PT[:, 2:3],
                                 start=True, stop=True)
                # ---------- ts, tt ----------
                nc.vector.scalar_tensor_tensor(
                    out=r3(TSa), in0=t3, scalar=1.0, in1=r3(s),
                    op0=OP.mult, op1=OP.mult, accum_out=PT[:, 3:4])
                nc.scalar.activation(r3(TSb), t3, AF.Square,
                                     accum_out=PT[:, 4:5])
                nc.tensor.matmul(ps_dots[:, 3:5], ones[:, :], PT[:, 3:5],
                                 start=True, stop=True)
                # ---------- C2 / FC4 / FC3 ----------
                ts_(S(C2), ps_dots[:, 2:3], THR2, None, OP.is_le)
                ts_(S(NOTC2), S(C2), -1.0, 1.0, OP.mult, OP.add)
                tt_(S(FC4), S(FNR), S(NOTC2), OP.mult)
                tt_(S(FC3), S(FNR), S(C2), OP.mult)
                ts_(S(NOTFC4), S(FC4), -1.0, 1.0, OP.mult, OP.add)
                # ---------- omega ----------
                ts_(S(TTS), ps_dots[:, 4:5], S(FC4), S(NOTFC4),
                    OP.mult, OP.add)
                nc.vector.reciprocal(S(RECB), S(TTS))
                tt_(S(OMEGA), ps_dots[:, 3:4], S(RECB), OP.mult)
                ts_(S(OMEGAX), S(OMEGA), S(FC4), None, OP.mult)
                ts_(S(NOMEGAX), S(OMEGAX), -1.0, None, OP.mult)
                # ---------- u = p - omegax*v (for p') ----------
                nc.vector.scalar_tensor_tensor(
                    out=r3(u), in0=v3, scalar=S(NOMEGAX), in1=r3(pcur),
                    op0=OP.mult, op1=OP.add)
                # ---------- r' = s - omegax*t ----------
                nc.vector.scalar_tensor_tensor(
                    out=r3(r), in0=t3, scalar=S(NOMEGAX), in1=r3(s),
                    op0=OP.mult, op1=OP.add)
                # ---------- x += alphax*p + omegax*s (Act scale + Pool add,
                #            off critical path) ----------
                nc.scalar.activation(TSc[:, :], pcur[:, :], AF.Identity,
                                     scale=S(ALPHAX))
                nc.gpsimd.tensor_add(x[:, :], x[:, :], TSc[:, :])
                nc.scalar.activation(TSd[:, :], s[:, :], AF.Identity,
                                     scale=S(OMEGAX))
                nc.gpsimd.tensor_add(x[:, :], x[:, :], TSd[:, :])
                # ---------- rho' = <r,r0>, rr = <r,r> ----------
                nc.vector.scalar_tensor_tensor(
                    out=TSa[:, :], in0=r[:, :], scalar=1.0, in1=r0[:, :],
                    op0=OP.mult, op1=OP.mult, accum_out=PT[:, 5:6])
                nc.scalar.activation(TSb[:, :], r[:, :], AF.Square,
                                     accum_out=PT[:, 6:7])
                nc.tensor.matmul(ps_dots[:, 5:7], ones[:, :], PT[:, 5:7],
                                 start=True, stop=True)
                # ---------- beta ----------
                ts_(S(OMS), S(OMEGAX), S(NOTFC4), None, OP.add)
                nc.vector.reciprocal(S(RECC), S(OMS))
                ts_(S(DENS), S(RHO), S(FC4), S(NOTFC4), OP.mult, OP.add)
                nc.vector.reciprocal(S(RECD), S(DENS))
                tt_(S(Q1), S(ALPHA), S(RECC), OP.mult)
                tt_(S(Q2), ps_dots[:, 5:6], S(RECD), OP.mult)
                tt_(S(BETA), S(Q1), S(Q2), OP.mult)
                ts_(S(BETAX), S(BETA), S(FC4), None, OP.mult)
                # ---------- p' = r + betax*u (Act scale + Pool add) ----------
                nc.scalar.activation(TSc[:, :], u[:, :], AF.Identity,
                                     scale=S(BETAX))
                nc.gpsimd.tensor_add(pnxt[:, :], r[:, :], TSc[:, :])
                # ---------- scalar state for next iter ----------
                nc.vector.copy_predicated(S(RHO), S(FNR).bitcast(u32),
                                          ps_dots[:, 5:6])
                nc.vector.copy_predicated(S(RABS2), S(FNR).bitcast(u32),
                                          ps_dots[:, 6:7])
                ts_(S(CONV), S(RABS2), THR2, None, OP.is_gt)
                ts_(S(NOTCONV), S(CONV), -1.0, 1.0, OP.mult, OP.add)
                tt_(S(FPFIX), S(FC3), S(NOTCONV), OP.add)
                # ---------- p fixup when frozen/C3 (rare/never) ----------
                for reg in regs_fix:
                    nc.reg_load(reg, SC[0:1, FPFIX:FPFIX + 1].bitcast(u32))
                with tc.If(nc.snap(regs_fix, donate=True) > 0):
                    nc.vector.tensor_copy(pnxt[:, :], pcur[:, :])

                pcur, pnxt = pnxt, pcur

            nc.sync.dma_start(xout, x[:, :])
    nc.compile()
    return nc


# ======================= public entry point =======================

_CACHE = {}


def kernel(V, mask1, mask2):
    B, C = V.shape[0], V.shape[1]
    assert (B, C) == (8, 1) and V.shape[2:] == (N, N)
    if "nc" not in _CACHE:
        _CACHE["nc"] = build_nc()
    nc = _CACHE["nc"]

    mats = make_mats()
    in_maps = []
    for b in range(B):
        h = host_prepare(np.asarray(V[b, 0], F32), np.asarray(mask1[b, 0], F32),
                         np.asarray(mask2[b, 0], F32))
        scal = np.zeros((P, 8), F32)
        scal[:, 0] = h["rho0"]                    # RHO
        scal[:, 1] = h["rho0"]                    # RABS2
        scal[:, 2] = h["rho0"]                    # R0SQ
        scal[:, 3] = F32(h["rho0"] * F32(EPS2))   # KEPS
        scal[:, 4] = h["c"]                       # CC
        scal[:, 5] = 1.0                          # CONV
        scal[:, 6] = 0.0                          # NOTCONV
        in_maps.append({
            "cC": to_dev(h["cC"]), "cU": to_dev(h["cU"]),
            "cD": to_dev(h["cD"]), "cL": to_dev(h["cL"]),
            "cR": to_dev(h["cR"]), "x0": to_dev(h["x0"]),
            "p0": to_dev(h["p0"]), "scal": scal, "mats": mats,
            "ones": np.ones((P, P), F32), "zcol": np.zeros((P, 1), F32),
        })

    res = bass_utils.run_bass_kernel_spmd(nc, in_maps, core_ids=list(range(8)))
    out = np.empty((B, C, N, N), F32)
    for b in range(B):
        out[b, 0] = res.results[b]["xout"].reshape(N, N)
    return out


if __name__ == "__main__":
    rng = np.random.default_rng(0)
    V = rng.random((8, 1, N, N), F32)
    m1 = rng.random((8, 1, N, N), F32)
    m2 = rng.random((8, 1, N, N), F32)
    out = kernel(V, m1, m2)
    print("kernel ran:", out.shape, out.dtype, float(np.abs(out).mean()))


# revision 8
# speedup vs baseline: 1.3950x; 1.3950x over previous
"""BiCGSTAB solver for nn_BiCG_Net on 8 TRN2 NeuronCores (pure data parallel).

v3: each core solves one (b,c) 384x384 5-point stencil system, KMAX=30
iterations, SBUF/PSUM-resident.

Layout: grid row i at (partition i//3, row i%3); free index f = 384*(i%3)+j;
to_dev is reshape(128, 1152).

apply_A: host pre-shifts neighbor coefficients so the five elementwise
products are offset-free fp16 ops (DVE 2x mode); shifts + the 5-term sum are
15 fp16 PE matmuls (identity/shift matrices) accumulating into fp32 PSUM.
Edge (symmetric-pad) terms are folded into the center coefficient.

Scalar algebra: sigma = <p, A^T r0> (w precomputed) so alpha is ready before
v lands; rho' = rho - alphax*sigma - omegax*<t,r0>; ||r'||^2 = ss -
2*omegax*ts + omegax^2*tt. s is computed speculatively with unmasked alpha;
the (never-taken in practice) restart branch repairs r/r0/w/s/t/dots after
the fact.
"""

import numpy as np

import concourse.bass as bass
import concourse.bacc as bacc
import concourse.mybir as mybir
import concourse.tile as tile
from concourse import bass_utils

F32 = np.float32
F16 = np.float16
N = 384
P = 128
RPB = 3
W = RPB * N        # 1152
KMAX = 30
EPS = 1e-9
THR2 = float(F32(EPS * N * N)) ** 2
EPS2 = float(F32(EPS)) ** 2

ET = mybir.EngineType

# ---------------- scalar slots in SC[128, NSLOT] ----------------
# first 8 loaded from host scal[P, 8]
(RHO, RABS2, KEPS, CC, CONV, NOTCONV, PAD0, PAD1,
 SGC, SS2, VVK, RESNEG, RESP, RES, FR1, FNR, RECA, ALPHA, NALPHA,
 ALPHAX, C2, NOTC2, FC4, FC3, NOTFC4, TTS, RECB, OMEGA, OMEGAX,
 NOMEGAX, OMS, RECC, DENS, RECD, E1, E2, E3, RHO2, Q1, Q2, BETA, BETAX,
 NBOX, SSS, G1, G2, G3, RR2, FPFIX) = range(49)
NSLOT = 49


# ======================= host-side precompute =======================

def _sym_pad2(a):
    return np.pad(a, ((1, 1), (1, 1)), mode='symmetric')


def host_prepare(V, mask1, mask2):
    Vt = np.ascontiguousarray(V.T)
    m1 = np.ascontiguousarray(mask1.T)
    m2 = np.ascontiguousarray(mask2.T)
    Vp = (_sym_pad2(Vt) + F32(1.0)).astype(F32)
    m1p = _sym_pad2(m1).astype(F32)
    m2p = _sym_pad2(m2).astype(F32)

    d1r = ((Vp[1:, :] - Vp[:-1, :]) / (F32(0.5) * (Vp[1:, :] + Vp[:-1, :]))).astype(F32)
    d2r = ((Vp[:, 1:] - Vp[:, :-1]) / (F32(0.5) * (Vp[:, 1:] + Vp[:, :-1]))).astype(F32)
    d1 = np.zeros((N + 2, N + 2), F32)
    d1[:N + 1, 1:N + 1] = d1r[:, 1:N + 1]
    d1 = (d1 * m1p).astype(F32)
    d2 = np.zeros((N + 2, N + 2), F32)
    d2[1:N + 1, :N + 1] = d2r[1:N + 1, :]
    d2 = (d2 * m2p).astype(F32)
    rx = F32(5.0)
    rxx = F32(10.0)
    dd1 = (np.pad(d1, ((1, 0), (0, 0)))[:-1, :] - d1).astype(F32)
    dd2 = (np.pad(d2, ((0, 0), (1, 0)))[:, :-1] - d2).astype(F32)
    boo = (F32(1.0) + F32(2.0) * (rxx + rxx) - rx * dd1 - rx * dd2)[1:N + 1, 1:N + 1].astype(F32)
    bpo = (-rxx + rx * d1[1:N + 1, 1:N + 1]).astype(F32)
    bop = (-rxx + rx * d2[1:N + 1, 1:N + 1]).astype(F32)
    bmo = (-rxx - rx * d1[:N, 1:N + 1]).astype(F32)
    bom = (-rxx - rx * d2[1:N + 1, :N]).astype(F32)

    # forward-stencil coefficients, edge-folded + pre-shifted
    cC = boo.copy()
    cC[0, :] += bmo[0, :]
    cC[N - 1, :] += bpo[N - 1, :]
    cC[:, 0] += bom[:, 0]
    cC[:, N - 1] += bop[:, N - 1]
    cU = np.zeros((N, N), F32); cU[:N - 1, :] = bmo[1:, :]
    cD = np.zeros((N, N), F32); cD[1:, :] = bpo[:N - 1, :]
    cL = np.zeros((N, N), F32); cL[:, :N - 1] = bom[:, 1:]
    cR = np.zeros((N, N), F32); cR[:, 1:] = bop[:, :N - 1]
    # transpose-stencil coefficients (for w = A^T r0), same q-form
    gC = cC
    gU = np.zeros((N, N), F32); gU[:N - 1, :] = cD[1:, :]
    gD = np.zeros((N, N), F32); gD[1:, :] = cU[:N - 1, :]
    gL = np.zeros((N, N), F32); gL[:, :N - 1] = cR[:, 1:]
    gR = np.zeros((N, N), F32); gR[:, 1:] = cL[:, :N - 1]

    c = F32(np.mean(V, dtype=F32) + F32(1.0))
    ax0 = ((((boo * c + bmo * c) + bom * c) + bop * c) + bpo * c).astype(F32)
    p0 = (c - ax0).astype(F32)
    x0 = np.full((N, N), c, F32)
    rho0 = F32(np.sum((p0 * p0).astype(F32), dtype=F32))
    return dict(cC=cC, cU=cU, cD=cD, cL=cL, cR=cR,
                gC=gC, gU=gU, gD=gD, gL=gL, gR=gR,
                x0=x0, p0=p0, c=c, rho0=rho0)


def to16(a):
    return np.ascontiguousarray(a.reshape(P, W).astype(F16))


def make_mats():
    I = np.eye(P, dtype=F16)
    Su = np.eye(P, k=1).astype(F16)   # out[m] = in[m-1], out[0] = 0
    Sd = np.eye(P, k=-1).astype(F16)  # out[m] = in[m+1], out[127] = 0
    return np.stack([I, Su, Sd])


# ======================= device program =======================

def build_nc(kmax=KMAX):
    nc = bacc.Bacc("TRN2", debug=False, num_devices=8)
    dt = mybir.dt.float32
    f16 = mybir.dt.float16
    u32 = mybir.dt.uint32
    OP = mybir.AluOpType
    AF = mybir.ActivationFunctionType

    din = {}
    for nm in ("cC", "cU", "cD", "cL", "cR", "gC", "gU", "gD", "gL", "gR"):
        din[nm] = nc.dram_tensor(nm, [P, W], f16, kind="ExternalInput").ap()
    x0_in = nc.dram_tensor("x0", [P, W], dt, kind="ExternalInput").ap()
    p0_in = nc.dram_tensor("p0", [P, W], f16, kind="ExternalInput").ap()
    r0_in = nc.dram_tensor("r0i", [P, W], dt, kind="ExternalInput").ap()
    scal_in = nc.dram_tensor("scal", [P, 8], dt, kind="ExternalInput").ap()
    mats_in = nc.dram_tensor("mats", [3, P, P], f16, kind="ExternalInput").ap()
    ones_in = nc.dram_tensor("ones", [P, P], dt, kind="ExternalInput").ap()
    zcol_in = nc.dram_tensor("zcol", [P, 1], f16, kind="ExternalInput").ap()
    xout = nc.dram_tensor("xout", [P, W], dt, kind="ExternalOutput").ap()

    with tile.TileContext(nc) as tc:
        import contextlib
        with contextlib.ExitStack() as ctx:
            big = ctx.enter_context(tc.tile_pool(name="big", bufs=1))
            small = ctx.enter_context(tc.tile_pool(name="small", bufs=1))
            psum = ctx.enter_context(tc.tile_pool(name="psum", bufs=1, space="PSUM"))

            cf = {nm: big.tile([P, W], f16, tag=nm, name=nm) for nm in
                  ("cC", "cU", "cD", "cL", "cR", "gC", "gU", "gD", "gL", "gR")}
            x = big.tile([P, W], dt, tag="x")
            r = big.tile([P, W], dt, tag="r")
            r0 = big.tile([P, W], dt, tag="r0")
            w = big.tile([P, W], dt, tag="w")
            pA = big.tile([P, W], f16, tag="pA")
            pB = big.tile([P, W], f16, tag="pB")
            s = big.tile([P, W], f16, tag="s")
            vb = big.tile([P, W], f16, tag="vb")
            tb = big.tile([P, W], f16, tag="tb")
            g = big.tile([P, W], f16, tag="g")
            TS16 = big.tile([P, W], f16, tag="TS16")   # fp16 dot scratch
            TSa = big.tile([P, W], dt, tag="TSa")      # fp32 dot scratch
            TSc = big.tile([P, W], dt, tag="TSc")      # scratch
            TSd = big.tile([P, W], dt, tag="TSd")
            qs = []
            for b in range(2):
                qc = big.tile([P, W], f16, tag=f"qc{b}")
                qu = big.tile([P, W], f16, tag=f"qu{b}")
                qd = big.tile([P, W], f16, tag=f"qd{b}")
                qL = big.tile([P, W + 1], f16, tag=f"qL{b}")
                qR = big.tile([P, W + 1], f16, tag=f"qR{b}")
                qs.append((qc, qu, qd, qL, qR))

            SC = small.tile([P, NSLOT], dt, tag="SC")
            PT = small.tile([P, 8], dt, tag="PT")
            I_ = small.tile([P, P], f16, tag="I_")
            Su_ = small.tile([P, P], f16, tag="Su_")
            Sd_ = small.tile([P, P], f16, tag="Sd_")
            ones = small.tile([P, P], dt, tag="ones")

            vps = psum.tile([P, 3 * 512], dt, tag="vps")
            tps = psum.tile([P, 3 * 512], dt, tag="tps")
            ps_dots = psum.tile([P, 8], dt, tag="ps_dots")

            def S(k):
                return SC[:, k:k + 1]

            def r3(t):
                return t[:].rearrange("p (g w) -> p g w", g=RPB)

            def p3(t):
                return t[:].rearrange("p (g w) -> p g w", g=RPB)[:, :, 0:N]

            v3 = p3(vps)
            t3 = p3(tps)

            def ts_(out, in0, s1, s2, op0, op1=None):
                kw = {} if op1 is None else {"op1": op1}
                nc.vector.tensor_scalar(out=out, in0=in0, scalar1=s1,
                                        scalar2=s2, op0=op0, **kw)

            def tt_(out, in0, in1, op):
                nc.vector.tensor_tensor(out=out, in0=in0, in1=in1, op=op)

            def products(z, qset, tr=False):
                """q = coeff * z, offset-free. DVE: qc,qd,qL; Pool: qR,qu."""
                pre = "g" if tr else "c"
                qc, qu, qd, qL, qR = qset
                nc.vector.tensor_mul(qc[:, :], cf[pre + "C"][:, :], z[:, :])
                nc.gpsimd.tensor_mul(qR[:, 0:W], cf[pre + "R"][:, :], z[:, :])
                nc.vector.tensor_mul(qd[:, :], cf[pre + "D"][:, :], z[:, :])
                nc.gpsimd.tensor_mul(qu[:, :], cf[pre + "U"][:, :], z[:, :])
                nc.vector.tensor_mul(qL[:, 1:W + 1], cf[pre + "L"][:, :], z[:, :])

            def stencil_mm(qset, outp):
                qc, qu, qd, qL, qR = qset
                o3 = outp[:].rearrange("p (g w) -> p g w", g=RPB)

                def mm(k, lhs, rhs_ap, start, stop):
                    nc.tensor.matmul(o3[:, k, 0:N], lhs[:, :], rhs_ap,
                                     start=start, stop=stop)
                for k in range(RPB):
                    ck = k * N
                    mm(k, I_, qc[:, ck:ck + N], True, False)
                    mm(k, I_, qR[:, ck + 1:ck + N + 1], False, False)
                    if k < RPB - 1:
                        mm(k, I_, qd[:, ck + N:ck + 2 * N], False, False)
                    else:
                        mm(k, Sd_, qd[:, 0:N], False, False)
                    mm(k, I_, qL[:, ck:ck + N], False, False)
                    if k == 0:
                        mm(k, Su_, qu[:, 2 * N:3 * N], False, True)
                    else:
                        mm(k, I_, qu[:, ck - N:ck], False, True)

            def dots_after_t():
                """tb = fp16(t); ts, tt, tr0 dots + reduce."""
                nc.scalar.activation(r3(tb), t3, AF.Copy)
                nc.vector.scalar_tensor_tensor(
                    out=TS16[:, :], in0=tb[:, :], scalar=1.0, in1=s[:, :],
                    op0=OP.mult, op1=OP.mult, accum_out=PT[:, 3:4])
                nc.scalar.activation(TSd[:, :], tb[:, :], AF.Square,
                                     accum_out=PT[:, 4:5])
                nc.vector.scalar_tensor_tensor(
                    out=TSa[:, :], in0=tb[:, :], scalar=1.0, in1=r0[:, :],
                    op0=OP.mult, op1=OP.mult, accum_out=PT[:, 5:6])
                nc.tensor.matmul(ps_dots[:, 3:6], ones[:, :], PT[:, 3:6],
                                 start=True, stop=True)

            # ---------------- loads / prologue ----------------
            nc.sync.dma_start(SC[:, 0:8], scal_in)
            for nm in cf:
                nc.sync.dma_start(cf[nm][:, :], din[nm])
            nc.sync.dma_start(x[:, :], x0_in)
            nc.sync.dma_start(pA[:, :], p0_in)
            nc.sync.dma_start(r[:, :], r0_in)
            nc.sync.dma_start(r0[:, :], r0_in)
            for i, t_ in enumerate((I_, Su_, Sd_)):
                nc.sync.dma_start(t_[:, :], mats_in[i])
            nc.sync.dma_start(ones[:, :], ones_in)
            for b in range(2):
                nc.sync.dma_start(qs[b][3][:, 0:1], zcol_in)
                nc.sync.dma_start(qs[b][4][:, W:W + 1], zcol_in)
            # w = A^T r0
            products(r0, qs[0], tr=True)
            stencil_mm(qs[0], vps)
            nc.scalar.activation(r3(w), v3, AF.Copy)

            regs_r1 = nc.alloc_registers(
                "fr1", bass.OrderedSet([ET.DVE, ET.Pool, ET.Activation, ET.PE]))
            regs_fix = nc.alloc_registers("ffix", bass.OrderedSet([ET.DVE]))

            pcur, pnxt = pA, pB
            for it in range(kmax):
                # ---------- sigma = <p, w> (ready before v) ----------
                nc.vector.scalar_tensor_tensor(
                    out=TSa[:, :], in0=w[:, :], scalar=1.0, in1=pcur[:, :],
                    op0=OP.mult, op1=OP.mult, accum_out=PT[:, 0:1])
                nc.tensor.matmul(ps_dots[:, 0:1], ones[:, :], PT[:, 0:1],
                                 start=True, stop=True)
                # ---------- v = A(p) ----------
                products(pcur, qs[0])
                stencil_mm(qs[0], vps)
                # ---------- early alpha (speculative, unmasked) ----------
                nc.vector.tensor_copy(S(SGC), ps_dots[:, 0:1])
                ts_(S(RESNEG), ps_dots[:, 0:1], 0.0, None, OP.is_le)
                tt_(S(SS2), ps_dots[:, 0:1], S(SGC), OP.mult)
                nc.vector.reciprocal(S(RECA), S(SGC))
                ts_(S(ALPHA), S(RECA), S(RHO), None, OP.mult)
                ts_(S(NALPHA), S(ALPHA), -1.0, None, OP.mult)
                # ---------- vv (Act) + RES flags ----------
                nc.scalar.activation(r3(TSc), v3, AF.Square,
                                     accum_out=PT[:, 1:2])
                nc.tensor.matmul(ps_dots[:, 1:2], ones[:, :], PT[:, 1:2],
                                 start=True, stop=True)
                tt_(S(VVK), ps_dots[:, 1:2], S(KEPS), OP.mult)
                tt_(S(RESP), S(SS2), S(VVK), OP.is_le)
                tt_(S(RES), S(RESNEG), S(RESP), OP.max)
                tt_(S(FR1), S(CONV), S(RES), OP.mult)
                tt_(S(FNR), S(CONV), S(FR1), OP.subtract)
                # ---------- s = r - alpha*v (speculative) ----------
                nc.vector.scalar_tensor_tensor(
                    out=r3(s), in0=v3, scalar=S(NALPHA), in1=r3(r),
                    op0=OP.mult, op1=OP.add)
                # ---------- vb = fp16(v) (Act; for p' tail) ----------
                nc.scalar.activation(r3(vb), v3, AF.Copy)
                # ---------- t = A(s) ----------
                products(s, qs[1])
                stencil_mm(qs[1], tps)
                # ---------- ss (Act) ----------
                nc.scalar.activation(TSc[:, :], s[:, :], AF.Square,
                                     accum_out=PT[:, 2:3])
                nc.tensor.matmul(ps_dots[:, 2:3], ones[:, :], PT[:, 2:3],
                                 start=True, stop=True)
                # ---------- tb + dots ts, tt, tr0 ----------
                dots_after_t()
                # ---------- restart branch (rare): full repair ----------
                for reg in regs_r1:
                    nc.reg_load(reg, SC[0:1, FR1:FR1 + 1].bitcast(u32))
                with tc.If(nc.snap(regs_r1, donate=True) > 0):
                    products(x, qs[1])
                    stencil_mm(qs[1], tps)
                    nc.vector.tensor_scalar(out=r3(r), in0=t3, scalar1=-1.0,
                                            scalar2=S(CC), op0=OP.mult,
                                            op1=OP.add)
                    nc.scalar.copy(r0[:, :], r[:, :])
                    nc.vector.tensor_copy(s[:, :], r[:, :])
                    nc.vector.memset(S(ALPHA), 0.0)
                    nc.vector.memset(S(NALPHA), 0.0)
                    nc.scalar.activation(TSc[:, :], r[:, :], AF.Square,
                                         accum_out=PT[:, 7:8])
                    nc.tensor.matmul(ps_dots[:, 7:8], ones[:, :], PT[:, 7:8],
                                     start=True, stop=True)
                    nc.vector.tensor_copy(S(RHO), ps_dots[:, 7:8])
                    nc.vector.tensor_copy(S(RABS2), ps_dots[:, 7:8])
                    ts_(S(KEPS), ps_dots[:, 7:8], EPS2, None, OP.mult)
                    products(r0, qs[0], tr=True)
                    stencil_mm(qs[0], vps)
                    nc.scalar.activation(r3(w), v3, AF.Copy)
                    products(s, qs[1])
                    stencil_mm(qs[1], tps)
                    dots_after_t()
                # ---------- flags: C2 / FC4 / FC3 ----------
                ts_(S(ALPHAX), S(ALPHA), S(FNR), None, OP.mult)
                nc.vector.tensor_copy(S(SSS), ps_dots[:, 2:3])
                ts_(S(C2), ps_dots[:, 2:3], THR2, None, OP.is_le)
                ts_(S(NOTC2), S(C2), -1.0, 1.0, OP.mult, OP.add)
                tt_(S(FC4), S(FNR), S(NOTC2), OP.mult)
                tt_(S(FC3), S(FNR), S(C2), OP.mult)
                ts_(S(NOTFC4), S(FC4), -1.0, 1.0, OP.mult, OP.add)
                # ---------- omega ----------
                ts_(S(TTS), ps_dots[:, 4:5], S(FC4), S(NOTFC4),
                    OP.mult, OP.add)
                nc.vector.reciprocal(S(RECB), S(TTS))
                tt_(S(OMEGA), ps_dots[:, 3:4], S(RECB), OP.mult)
                ts_(S(OMEGAX), S(OMEGA), S(FC4), None, OP.mult)
                ts_(S(NOMEGAX), S(OMEGAX), -1.0, None, OP.mult)
                # ---------- beta (scalar recurrences) ----------
                ts_(S(OMS), S(OMEGAX), S(NOTFC4), None, OP.add)
                nc.vector.reciprocal(S(RECC), S(OMS))
                ts_(S(DENS), S(RHO), S(FC4), S(NOTFC4), OP.mult, OP.add)
                nc.vector.reciprocal(S(RECD), S(DENS))
                tt_(S(E1), S(ALPHAX), S(SGC), OP.mult)
                tt_(S(E2), S(RHO), S(E1), OP.subtract)
                ts_(S(E3), ps_dots[:, 5:6], S(OMEGAX), None, OP.mult)
                tt_(S(RHO2), S(E2), S(E3), OP.subtract)
                tt_(S(Q1), S(ALPHA), S(RECC), OP.mult)
                tt_(S(Q2), S(RHO2), S(RECD), OP.mult)
                tt_(S(BETA), S(Q1), S(Q2), OP.mult)
                ts_(S(BETAX), S(BETA), S(FC4), None, OP.mult)
                tt_(S(NBOX), S(BETAX), S(NOMEGAX), OP.mult)
                # ---------- r' = s - omegax*t ----------
                nc.vector.scalar_tensor_tensor(
                    out=r[:, :], in0=tb[:, :], scalar=S(NOMEGAX), in1=s[:, :],
                    op0=OP.mult, op1=OP.add)
                # ---------- p' = r' + betax*p + nbox*vb ----------
                ts_(g[:, :], pcur[:, :], S(BETAX), None, OP.mult)
                nc.vector.scalar_tensor_tensor(
                    out=g[:, :], in0=vb[:, :], scalar=S(NBOX), in1=g[:, :],
                    op0=OP.mult, op1=OP.add)
                tt_(pnxt[:, :], r[:, :], g[:, :], OP.add)
                # ---------- x += alphax*p + omegax*s (off path) ----------
                nc.scalar.activation(TSc[:, :], pcur[:, :], AF.Identity,
                                     scale=S(ALPHAX))
                nc.gpsimd.tensor_add(x[:, :], x[:, :], TSc[:, :])
                nc.scalar.activation(TSd[:, :], s[:, :], AF.Identity,
                                     scale=S(OMEGAX))
                nc.gpsimd.tensor_add(x[:, :], x[:, :], TSd[:, :])
                # ---------- scalar state for next iter ----------
                tt_(S(G1), S(OMEGAX), ps_dots[:, 3:4], OP.mult)
                ts_(S(G2), S(G1), -2.0, S(SSS), OP.mult, OP.add)
                tt_(S(G3), S(OMEGAX), S(OMEGAX), OP.mult)
                ts_(S(E1), ps_dots[:, 4:5], S(G3), None, OP.mult)
                tt_(S(RR2), S(G2), S(E1), OP.add)
                nc.vector.copy_predicated(S(RHO), S(FNR).bitcast(u32),
                                          S(RHO2))
                nc.vector.copy_predicated(S(RABS2), S(FNR).bitcast(u32),
                                          S(RR2))
                ts_(S(CONV), S(RABS2), THR2, None, OP.is_gt)
                ts_(S(NOTCONV), S(CONV), -1.0, 1.0, OP.mult, OP.add)
                tt_(S(FPFIX), S(FC3), S(NOTCONV), OP.add)
                # ---------- p fixup when frozen/C3 (rare/never) ----------
                for reg in regs_fix:
                    nc.reg_load(reg, SC[0:1, FPFIX:FPFIX + 1].bitcast(u32))
                with tc.If(nc.snap(regs_fix, donate=True) > 0):
                    nc.vector.tensor_copy(pnxt[:, :], pcur[:, :])

                pcur, pnxt = pnxt, pcur

            nc.sync.dma_start(xout, x[:, :])
    nc.compile()
    return nc


# ======================= public entry point =======================

_CACHE = {}


def kernel(V, mask1, mask2):
    B, C = V.shape[0], V.shape[1]
    assert (B, C) == (8, 1) and V.shape[2:] == (N, N)
    if "nc" not in _CACHE:
        _CACHE["nc"] = build_nc()
    nc = _CACHE["nc"]

    mats = make_mats()
    in_maps = []
    for b in range(B):
        h = host_prepare(np.asarray(V[b, 0], F32), np.asarray(mask1[b, 0], F32),
                         np.asarray(mask2[b, 0], F32))
        scal = np.zeros((P, 8), F32)
        scal[:, 0] = h["rho0"]                    # RHO
        scal[:, 1] = h["rho0"]                    # RABS2
        scal[:, 2] = F32(h["rho0"] * F32(EPS2))   # KEPS
        scal[:, 3] = h["c"]                       # CC
        scal[:, 4] = 1.0                          # CONV
        scal[:, 5] = 0.0                          # NOTCONV
        im = {nm: to16(h[nm]) for nm in
              ("cC", "cU", "cD", "cL", "cR", "gC", "gU", "gD", "gL", "gR")}
        im["x0"] = np.ascontiguousarray(h["x0"].reshape(P, W))
        im["p0"] = to16(h["p0"])
        im["r0i"] = np.ascontiguousarray(h["p0"].reshape(P, W))
        im["scal"] = scal
        im["mats"] = mats
        im["ones"] = np.ones((P, P), F32)
        im["zcol"] = np.zeros((P, 1), F16)
        in_maps.append(im)

    res = bass_utils.run_bass_kernel_spmd(nc, in_maps, core_ids=list(range(8)))
    out = np.empty((B, C, N, N), F32)
    for b in range(B):
        out[b, 0] = res.results[b]["xout"].reshape(N, N)
    return out


if __name__ == "__main__":
    rng = np.random.default_rng(0)
    V = rng.random((8, 1, N, N), F32)
    m1 = rng.random((8, 1, N, N), F32)
    m2 = rng.random((8, 1, N, N), F32)
    out = kernel(V, m1, m2)
    print("kernel ran:", out.shape, out.dtype, float(np.abs(out).mean()))


# revision 11
# speedup vs baseline: 1.4308x; 1.0257x over previous
"""BiCGSTAB solver for nn_BiCG_Net on 8 TRN2 NeuronCores (pure data parallel).

v4: each core solves one (b,c) 384x384 5-point stencil system, KMAX=30
iterations, SBUF/PSUM-resident.

Layout: grid row i at (partition i//3, row i%3); free f = 384*(i%3)+j.

apply_A: host pre-shifts neighbor coefficients so the five elementwise
products are offset-free pure-fp16 ops (DVE 2x mode); shifts + sum are 15
fp16 PE matmuls accumulating into fp32 PSUM. Edge terms folded into the
center coefficient on the host.

dtype split (hw-measured): pure-fp16 no-accum DVE ops are 2x; any mixed or
accum op must be pure fp32 to avoid the DVE conversion path. So: products
and the r'/p' tail run fp16 (s/p have fp16 copies), reduction dots run pure
fp32, v/t accumulate in fp32 PSUM.

Scalar algebra: sigma = <p,w>, tr0 = <s,w> with w = A^T r0 (so both dots
avoid v/t and run early); rho' and ||r'||^2 via scalar recurrences. s is
speculative (unmasked alpha); the rare restart branch repairs everything.
"""

import numpy as np

import concourse.bass as bass
import concourse.bacc as bacc
import concourse.mybir as mybir
import concourse.tile as tile
from concourse import bass_utils

F32 = np.float32
F16 = np.float16
N = 384
P = 128
RPB = 3
W = RPB * N        # 1152
KMAX = 30
EPS = 1e-9
THR2 = float(F32(EPS * N * N)) ** 2
EPS2 = float(F32(EPS)) ** 2

ET = mybir.EngineType

# ---------------- scalar slots in SC[128, NSLOT] ----------------
(RHO, RABS2, KEPS, CC, CONV, NOTCONV, PAD0, PAD1,
 SGC, SS2, VVK, RESNEG, RESP, RES, FR1, FNR, RECA, ALPHA, NALPHA,
 ALPHAX, C2, NOTC2, FC4, FC3, NOTFC4, TTS, RECB, OMEGA, OMEGAX,
 NOMEGAX, OMS, RECC, DENS, RECD, E1, E2, E3, RHO2, Q1, Q2, BETA, BETAX,
 NBOX, SSS, G1, G2, G3, RR2, FPFIX) = range(49)
NSLOT = 49


# ======================= host-side precompute =======================

def _sym_pad2(a):
    return np.pad(a, ((1, 1), (1, 1)), mode='symmetric')


def host_prepare(V, mask1, mask2):
    Vt = np.ascontiguousarray(V.T)
    m1 = np.ascontiguousarray(mask1.T)
    m2 = np.ascontiguousarray(mask2.T)
    Vp = (_sym_pad2(Vt) + F32(1.0)).astype(F32)
    m1p = _sym_pad2(m1).astype(F32)
    m2p = _sym_pad2(m2).astype(F32)

    d1r = ((Vp[1:, :] - Vp[:-1, :]) / (F32(0.5) * (Vp[1:, :] + Vp[:-1, :]))).astype(F32)
    d2r = ((Vp[:, 1:] - Vp[:, :-1]) / (F32(0.5) * (Vp[:, 1:] + Vp[:, :-1]))).astype(F32)
    d1 = np.zeros((N + 2, N + 2), F32)
    d1[:N + 1, 1:N + 1] = d1r[:, 1:N + 1]
    d1 = (d1 * m1p).astype(F32)
    d2 = np.zeros((N + 2, N + 2), F32)
    d2[1:N + 1, :N + 1] = d2r[1:N + 1, :]
    d2 = (d2 * m2p).astype(F32)
    rx = F32(5.0)
    rxx = F32(10.0)
    dd1 = (np.pad(d1, ((1, 0), (0, 0)))[:-1, :] - d1).astype(F32)
    dd2 = (np.pad(d2, ((0, 0), (1, 0)))[:, :-1] - d2).astype(F32)
    boo = (F32(1.0) + F32(2.0) * (rxx + rxx) - rx * dd1 - rx * dd2)[1:N + 1, 1:N + 1].astype(F32)
    bpo = (-rxx + rx * d1[1:N + 1, 1:N + 1]).astype(F32)
    bop = (-rxx + rx * d2[1:N + 1, 1:N + 1]).astype(F32)
    bmo = (-rxx - rx * d1[:N, 1:N + 1]).astype(F32)
    bom = (-rxx - rx * d2[1:N + 1, :N]).astype(F32)

    cC = boo.copy()
    cC[0, :] += bmo[0, :]
    cC[N - 1, :] += bpo[N - 1, :]
    cC[:, 0] += bom[:, 0]
    cC[:, N - 1] += bop[:, N - 1]
    cU = np.zeros((N, N), F32); cU[:N - 1, :] = bmo[1:, :]
    cD = np.zeros((N, N), F32); cD[1:, :] = bpo[:N - 1, :]
    cL = np.zeros((N, N), F32); cL[:, :N - 1] = bom[:, 1:]
    cR = np.zeros((N, N), F32); cR[:, 1:] = bop[:, :N - 1]
    gC = cC
    gU = np.zeros((N, N), F32); gU[:N - 1, :] = cD[1:, :]
    gD = np.zeros((N, N), F32); gD[1:, :] = cU[:N - 1, :]
    gL = np.zeros((N, N), F32); gL[:, :N - 1] = cR[:, 1:]
    gR = np.zeros((N, N), F32); gR[:, 1:] = cL[:, :N - 1]

    c = F32(np.mean(V, dtype=F32) + F32(1.0))
    ax0 = ((((boo * c + bmo * c) + bom * c) + bop * c) + bpo * c).astype(F32)
    p0 = (c - ax0).astype(F32)
    x0 = np.full((N, N), c, F32)
    rho0 = F32(np.sum((p0 * p0).astype(F32), dtype=F32))
    return dict(cC=cC, cU=cU, cD=cD, cL=cL, cR=cR,
                gC=gC, gU=gU, gD=gD, gL=gL, gR=gR,
                x0=x0, p0=p0, c=c, rho0=rho0)


def to16(a):
    return np.ascontiguousarray(a.reshape(P, W).astype(F16))


def make_mats():
    I = np.eye(P, dtype=F16)
    Su = np.eye(P, k=1).astype(F16)
    Sd = np.eye(P, k=-1).astype(F16)
    return np.stack([I, Su, Sd])


# ======================= device program =======================

def build_nc(kmax=KMAX):
    nc = bacc.Bacc("TRN2", debug=False, num_devices=8)
    dt = mybir.dt.float32
    f16 = mybir.dt.float16
    u32 = mybir.dt.uint32
    OP = mybir.AluOpType
    AF = mybir.ActivationFunctionType

    din = {}
    for nm in ("cC", "cU", "cD", "cL", "cR", "gC", "gU", "gD", "gL", "gR"):
        din[nm] = nc.dram_tensor(nm, [P, W], f16, kind="ExternalInput").ap()
    x0_in = nc.dram_tensor("x0", [P, W], dt, kind="ExternalInput").ap()
    p0_in = nc.dram_tensor("p0", [P, W], f16, kind="ExternalInput").ap()
    r0_in = nc.dram_tensor("r0i", [P, W], dt, kind="ExternalInput").ap()
    scal_in = nc.dram_tensor("scal", [P, 8], dt, kind="ExternalInput").ap()
    mats_in = nc.dram_tensor("mats", [3, P, P], f16, kind="ExternalInput").ap()
    ones_in = nc.dram_tensor("ones", [P, P], dt, kind="ExternalInput").ap()
    zcol_in = nc.dram_tensor("zcol", [P, 1], f16, kind="ExternalInput").ap()
    xout = nc.dram_tensor("xout", [P, W], dt, kind="ExternalOutput").ap()

    with tile.TileContext(nc) as tc:
        import contextlib
        with contextlib.ExitStack() as ctx:
            big = ctx.enter_context(tc.tile_pool(name="big", bufs=1))
            small = ctx.enter_context(tc.tile_pool(name="small", bufs=1))
            psum = ctx.enter_context(tc.tile_pool(name="psum", bufs=1, space="PSUM"))

            cf = {nm: big.tile([P, W], f16, tag=nm, name=nm) for nm in
                  ("cC", "cU", "cD", "cL", "cR", "gC", "gU", "gD", "gL", "gR")}
            x = big.tile([P, W], dt, tag="x")
            r = big.tile([P, W], dt, tag="r")         # fp32 residual state
            r0 = big.tile([P, W], dt, tag="r0")
            w = big.tile([P, W], dt, tag="w")
            w16 = big.tile([P, W], f16, tag="w16")
            pA = big.tile([P, W], f16, tag="pA")
            pB = big.tile([P, W], f16, tag="pB")
            s16 = big.tile([P, W], f16, tag="s16")
            s32 = big.tile([P, W], dt, tag="s32")
            vb = big.tile([P, W], f16, tag="vb")
            tb = big.tile([P, W], f16, tag="tb")
            r16 = big.tile([P, W], f16, tag="r16")
            g = big.tile([P, W], f16, tag="g")
            TS16 = big.tile([P, W], f16, tag="TS16")
            TSa = big.tile([P, W], dt, tag="TSa")
            TSc = big.tile([P, W], dt, tag="TSc")
            TSd = big.tile([P, W], dt, tag="TSd")
            qs = []
            for b in range(2):
                qc = big.tile([P, W], f16, tag=f"qc{b}")
                qu = big.tile([P, W], f16, tag=f"qu{b}")
                qd = big.tile([P, W], f16, tag=f"qd{b}")
                qL = big.tile([P, W + 2], f16, tag=f"qL{b}")  # data at [2:W+2]
                qR = big.tile([P, W + 1], f16, tag=f"qR{b}")  # data at [0:W]
                qs.append((qc, qu, qd, qL, qR))

            SC = small.tile([P, NSLOT], dt, tag="SC")
            PT = small.tile([P, 8], dt, tag="PT")
            I_ = small.tile([P, P], f16, tag="I_")
            Su_ = small.tile([P, P], f16, tag="Su_")
            Sd_ = small.tile([P, P], f16, tag="Sd_")
            ones = small.tile([P, P], dt, tag="ones")

            vps = psum.tile([P, 3 * 512], dt, tag="vps")
            tps = psum.tile([P, 3 * 512], dt, tag="tps")
            ps_dots = psum.tile([P, 8], dt, tag="ps_dots")

            def S(k):
                return SC[:, k:k + 1]

            def r3(t):
                return t[:].rearrange("p (g w) -> p g w", g=RPB)

            def p3(t):
                return t[:].rearrange("p (g w) -> p g w", g=RPB)[:, :, 0:N]

            v3 = p3(vps)
            t3 = p3(tps)

            def ts_(out, in0, s1, s2, op0, op1=None):
                kw = {} if op1 is None else {"op1": op1}
                nc.vector.tensor_scalar(out=out, in0=in0, scalar1=s1,
                                        scalar2=s2, op0=op0, **kw)

            def tt_(out, in0, in1, op):
                nc.vector.tensor_tensor(out=out, in0=in0, in1=in1, op=op)

            def act(out, in_, func=None, bias=0.0, scale=1.0, accum=None):
                nc.scalar.activation(out, in_,
                                     func or mybir.ActivationFunctionType.Identity,
                                     bias=bias, scale=scale, accum_out=accum)

            def products(z, qset, tr=False):
                pre = "g" if tr else "c"
                qc, qu, qd, qL, qR = qset
                nc.vector.tensor_mul(qc[:, :], cf[pre + "C"][:, :], z[:, :])
                nc.gpsimd.tensor_mul(qR[:, 0:W], cf[pre + "R"][:, :], z[:, :])
                nc.vector.tensor_mul(qd[:, :], cf[pre + "D"][:, :], z[:, :])
                nc.gpsimd.tensor_mul(qu[:, :], cf[pre + "U"][:, :], z[:, :])
                nc.vector.tensor_mul(qL[:, 2:W + 2], cf[pre + "L"][:, :], z[:, :])

            def stencil_mm(qset, outp):
                qc, qu, qd, qL, qR = qset
                o3 = outp[:].rearrange("p (g w) -> p g w", g=RPB)

                def mm(k, lhs, rhs_ap, start, stop):
                    nc.tensor.matmul(o3[:, k, 0:N], lhs[:, :], rhs_ap,
                                     start=start, stop=stop)
                for k in range(RPB):
                    ck = k * N
                    mm(k, I_, qc[:, ck:ck + N], True, False)
                    mm(k, I_, qR[:, ck + 1:ck + N + 1], False, False)
                    if k < RPB - 1:
                        mm(k, I_, qd[:, ck + N:ck + 2 * N], False, False)
                    else:
                        mm(k, Sd_, qd[:, 0:N], False, False)
                    mm(k, I_, qL[:, ck + 1:ck + N + 1], False, False)
                    if k == 0:
                        mm(k, Su_, qu[:, 2 * N:3 * N], False, True)
                    else:
                        mm(k, I_, qu[:, ck - N:ck], False, True)

            def dots_after_t():
                """ts = <t,s32> (DVE), tt = <t,t> (Act), reduce [3:6]
                (tr0 already accumulated into PT5)."""
                nc.vector.scalar_tensor_tensor(
                    out=TSa[:, :], in0=t3, scalar=1.0, in1=r3(s32),
                    op0=OP.mult, op1=OP.mult, accum_out=PT[:, 3:4])
                nc.scalar.activation(r3(TSc), t3, AF.Square,
                                     accum_out=PT[:, 4:5])
                nc.scalar.activation(r3(tb), t3, AF.Copy)
                nc.tensor.matmul(ps_dots[:, 3:6], ones[:, :], PT[:, 3:6],
                                 start=True, stop=True)

            def tr0_dot():
                nc.vector.scalar_tensor_tensor(
                    out=TSa[:, :], in0=w[:, :], scalar=1.0, in1=s32[:, :],
                    op0=OP.mult, op1=OP.mult, accum_out=PT[:, 5:6])

            # ---------------- loads / prologue ----------------
            nc.sync.dma_start(SC[:, 0:8], scal_in)
            for nm in cf:
                nc.sync.dma_start(cf[nm][:, :], din[nm])
            nc.sync.dma_start(x[:, :], x0_in)
            nc.sync.dma_start(pA[:, :], p0_in)
            nc.sync.dma_start(r[:, :], r0_in)
            nc.sync.dma_start(r0[:, :], r0_in)
            for i, t_ in enumerate((I_, Su_, Sd_)):
                nc.sync.dma_start(t_[:, :], mats_in[i])
            nc.sync.dma_start(ones[:, :], ones_in)
            for b in range(2):
                nc.sync.dma_start(qs[b][3][:, 1:2], zcol_in)
                nc.sync.dma_start(qs[b][4][:, W:W + 1], zcol_in)
            # w = A^T r0 (r0 fp32: mixed products, prologue only)
            products(r0, qs[0], tr=True)
            stencil_mm(qs[0], vps)
            nc.scalar.activation(r3(w), v3, AF.Copy)
            nc.scalar.activation(r3(w16), v3, AF.Copy, scale=1.0 / 256.0)

            regs_r1 = nc.alloc_registers(
                "fr1", bass.OrderedSet([ET.DVE, ET.Pool, ET.Activation, ET.PE]))
            regs_fix = nc.alloc_registers("ffix", bass.OrderedSet([ET.DVE]))

            def restart_repair():
                """Body of the (never-taken in practice) restart branch."""
                products(x, qs[1])
                stencil_mm(qs[1], tps)
                nc.vector.tensor_scalar(out=r3(r), in0=t3, scalar1=-1.0,
                                        scalar2=S(CC), op0=OP.mult, op1=OP.add)
                nc.scalar.copy(r0[:, :], r[:, :])
                nc.vector.tensor_copy(s32[:, :], r[:, :])
                nc.vector.tensor_copy(s16[:, :], r[:, :])
                nc.vector.memset(S(ALPHA), 0.0)
                nc.vector.memset(S(NALPHA), 0.0)
                act(TSc[:, :], r[:, :], AF.Square, accum=PT[:, 7:8])
                nc.tensor.matmul(ps_dots[:, 7:8], ones[:, :], PT[:, 7:8],
                                 start=True, stop=True)
                nc.vector.tensor_copy(S(RHO), ps_dots[:, 7:8])
                nc.vector.tensor_copy(S(RABS2), ps_dots[:, 7:8])
                ts_(S(KEPS), ps_dots[:, 7:8], EPS2, None, OP.mult)
                products(r0, qs[0], tr=True)
                stencil_mm(qs[0], vps)
                nc.scalar.activation(r3(w), v3, AF.Copy)
                nc.scalar.activation(r3(w16), v3, AF.Copy, scale=1.0 / 256.0)
                tr0_dot()
                products(s16, qs[1])
                stencil_mm(qs[1], tps)
                dots_after_t()

            pcur, pnxt = pA, pB
            for it in range(kmax):
                # ---------- sigma = <p16, w16> (DVE prod + Act accum) ------
                nc.vector.tensor_mul(TS16[:, :], pcur[:, :], w16[:, :])
                act(TSc[:, :], TS16[:, :], accum=PT[:, 0:1])
                nc.tensor.matmul(ps_dots[:, 0:1], ones[:, :], PT[:, 0:1],
                                 start=True, stop=True)
                # ---------- v = A(p) ----------
                products(pcur, qs[0])
                stencil_mm(qs[0], vps)
                # ---------- early alpha (speculative) ----------
                ts_(S(SGC), ps_dots[:, 0:1], 256.0, None, OP.mult)
                ts_(S(RESNEG), ps_dots[:, 0:1], 0.0, None, OP.is_le)
                nc.vector.reciprocal(S(RECA), S(SGC))
                ts_(S(ALPHA), S(RECA), S(RHO), None, OP.mult)
                ts_(S(NALPHA), S(ALPHA), -1.0, None, OP.mult)
                # ---------- vv (Act) + RES flags ----------
                act(S(SS2), S(SGC), AF.Square)
                nc.scalar.activation(r3(TSc), v3, AF.Square,
                                     accum_out=PT[:, 1:2])
                nc.tensor.matmul(ps_dots[:, 1:2], ones[:, :], PT[:, 1:2],
                                 start=True, stop=True)
                act(S(VVK), ps_dots[:, 1:2], scale=S(KEPS))
                tt_(S(RESP), S(SS2), S(VVK), OP.is_le)
                tt_(S(RES), S(RESNEG), S(RESP), OP.max)
                tt_(S(FR1), S(CONV), S(RES), OP.mult)
                tt_(S(FNR), S(CONV), S(FR1), OP.subtract)
                # ---------- s (speculative): fp16 then fp32 ----------
                nc.vector.scalar_tensor_tensor(
                    out=r3(s16), in0=v3, scalar=S(NALPHA), in1=r3(r),
                    op0=OP.mult, op1=OP.add)
                # ---------- t = A(s) ----------
                products(s16, qs[1])
                stencil_mm(qs[1], tps)
                nc.vector.scalar_tensor_tensor(
                    out=r3(s32), in0=v3, scalar=S(NALPHA), in1=r3(r),
                    op0=OP.mult, op1=OP.add)
                # ---------- tr0 = <s32, w> (early, off tail) ----------
                tr0_dot()
                # ---------- vb (Act, for p' tail) ----------
                nc.scalar.activation(r3(vb), v3, AF.Copy)
                # ---------- ss (Act) ----------
                nc.scalar.activation(TSd[:, :], s32[:, :], AF.Square,
                                     accum_out=PT[:, 2:3])
                nc.tensor.matmul(ps_dots[:, 2:3], ones[:, :], PT[:, 2:3],
                                 start=True, stop=True)
                # ---------- restart branch (rare) ----------
                for reg in regs_r1:
                    nc.reg_load(reg, SC[0:1, FR1:FR1 + 1].bitcast(u32))
                with tc.If(nc.snap(regs_r1, donate=True) > 0):
                    restart_repair()
                # ---------- ts/tt dots + reduce ----------
                dots_after_t()
                # ---------- flags C2/FC4/FC3 (Act) + ALPHAX ----------
                act(S(ALPHAX), S(ALPHA), scale=S(FNR))
                nc.vector.tensor_copy(S(SSS), ps_dots[:, 2:3])
                ts_(S(C2), ps_dots[:, 2:3], THR2, None, OP.is_le)
                act(S(NOTC2), S(C2), scale=-1.0, bias=1.0)
                act(S(FC4), S(FNR), scale=S(NOTC2))
                act(S(FC3), S(FNR), scale=S(C2))
                act(S(NOTFC4), S(FC4), scale=-1.0, bias=1.0)
                # ---------- omega (DVE chain) ----------
                ts_(S(TTS), ps_dots[:, 4:5], S(FC4), S(NOTFC4),
                    OP.mult, OP.add)
                nc.vector.reciprocal(S(RECB), S(TTS))
                tt_(S(OMEGA), ps_dots[:, 3:4], S(RECB), OP.mult)
                ts_(S(OMEGAX), S(OMEGA), S(FC4), None, OP.mult)
                ts_(S(NOMEGAX), S(OMEGAX), -1.0, None, OP.mult)
                # ---------- r16 = s16 - omegax*tb (fast tail) ----------
                nc.vector.scalar_tensor_tensor(
                    out=r16[:, :], in0=tb[:, :], scalar=S(NOMEGAX),
                    in1=s16[:, :], op0=OP.mult, op1=OP.add)
                # ---------- beta (DVE chain) ----------
                ts_(S(OMS), S(OMEGAX), S(NOTFC4), None, OP.add)
                nc.vector.reciprocal(S(RECC), S(OMS))
                ts_(S(DENS), S(RHO), S(FC4), S(NOTFC4), OP.mult, OP.add)
                nc.vector.reciprocal(S(RECD), S(DENS))
                tt_(S(E1), S(ALPHAX), S(SGC), OP.mult)
                tt_(S(E2), S(RHO), S(E1), OP.subtract)
                ts_(S(E3), ps_dots[:, 5:6], S(OMEGAX), None, OP.mult)
                tt_(S(RHO2), S(E2), S(E3), OP.subtract)
                tt_(S(Q1), S(ALPHA), S(RECC), OP.mult)
                tt_(S(Q2), S(RHO2), S(RECD), OP.mult)
                tt_(S(BETA), S(Q1), S(Q2), OP.mult)
                ts_(S(BETAX), S(BETA), S(FC4), None, OP.mult)
                tt_(S(NBOX), S(BETAX), S(NOMEGAX), OP.mult)
                # ---------- p' = r16 + betax*p + nbox*vb (pure fp16) -------
                ts_(g[:, :], pcur[:, :], S(BETAX), None, OP.mult)
                nc.vector.scalar_tensor_tensor(
                    out=g[:, :], in0=vb[:, :], scalar=S(NBOX), in1=g[:, :],
                    op0=OP.mult, op1=OP.add)
                tt_(pnxt[:, :], r16[:, :], g[:, :], OP.add)
                # ---------- r32 = s32 - omegax*t (Act+Pool, off path) ------
                nc.scalar.activation(TSc[:, :], tb[:, :], AF.Identity,
                                     scale=S(NOMEGAX))
                nc.gpsimd.tensor_add(r[:, :], s32[:, :], TSc[:, :])
                # ---------- x += alphax*p + omegax*s (off path) ----------
                nc.scalar.activation(TSd[:, :], pcur[:, :], AF.Identity,
                                     scale=S(ALPHAX))
                nc.gpsimd.tensor_add(x[:, :], x[:, :], TSd[:, :])
                nc.scalar.activation(TSd[:, :], s16[:, :], AF.Identity,
                                     scale=S(OMEGAX))
                nc.gpsimd.tensor_add(x[:, :], x[:, :], TSd[:, :])
                # ---------- scalar state for next iter ----------
                tt_(S(G1), S(OMEGAX), ps_dots[:, 3:4], OP.mult)
                ts_(S(G2), S(G1), -2.0, S(SSS), OP.mult, OP.add)
                tt_(S(G3), S(OMEGAX), S(OMEGAX), OP.mult)
                ts_(S(E1), ps_dots[:, 4:5], S(G3), None, OP.mult)
                tt_(S(RR2), S(G2), S(E1), OP.add)
                nc.vector.copy_predicated(S(RHO), S(FNR).bitcast(u32),
                                          S(RHO2))
                nc.vector.copy_predicated(S(RABS2), S(FNR).bitcast(u32),
                                          S(RR2))
                ts_(S(CONV), S(RABS2), THR2, None, OP.is_gt)
                ts_(S(NOTCONV), S(CONV), -1.0, 1.0, OP.mult, OP.add)
                tt_(S(FPFIX), S(FC3), S(NOTCONV), OP.add)
                # ---------- p fixup (rare/never) ----------
                for reg in regs_fix:
                    nc.reg_load(reg, SC[0:1, FPFIX:FPFIX + 1].bitcast(u32))
                with tc.If(nc.snap(regs_fix, donate=True) > 0):
                    nc.vector.tensor_copy(pnxt[:, :], pcur[:, :])

                pcur, pnxt = pnxt, pcur

            nc.sync.dma_start(xout, x[:, :])
    nc.compile()
    return nc


# ======================= public entry point =======================

_CACHE = {}


def kernel(V, mask1, mask2):
    B, C = V.shape[0], V.shape[1]
    assert (B, C) == (8, 1) and V.shape[2:] == (N, N)
    if "nc" not in _CACHE:
        _CACHE["nc"] = build_nc()
    nc = _CACHE["nc"]

    mats = make_mats()
    in_maps = []
    for b in range(B):
        h = host_prepare(np.asarray(V[b, 0], F32), np.asarray(mask1[b, 0], F32),
                         np.asarray(mask2[b, 0], F32))
        scal = np.zeros((P, 8), F32)
        scal[:, 0] = h["rho0"]
        scal[:, 1] = h["rho0"]
        scal[:, 2] = F32(h["rho0"] * F32(EPS2))
        scal[:, 3] = h["c"]
        scal[:, 4] = 1.0
        scal[:, 5] = 0.0
        im = {nm: to16(h[nm]) for nm in
              ("cC", "cU", "cD", "cL", "cR", "gC", "gU", "gD", "gL", "gR")}
        im["x0"] = np.ascontiguousarray(h["x0"].reshape(P, W))
        im["p0"] = to16(h["p0"])
        im["r0i"] = np.ascontiguousarray(h["p0"].reshape(P, W))
        im["scal"] = scal
        im["mats"] = mats
        im["ones"] = np.ones((P, P), F32)
        im["zcol"] = np.zeros((P, 1), F16)
        in_maps.append(im)

    res = bass_utils.run_bass_kernel_spmd(nc, in_maps, core_ids=list(range(8)))
    out = np.empty((B, C, N, N), F32)
    for b in range(B):
        out[b, 0] = res.results[b]["xout"].reshape(N, N)
    return out


if __name__ == "__main__":
    rng = np.random.default_rng(0)
    V = rng.random((8, 1, N, N), F32)
    m1 = rng.random((8, 1, N, N), F32)
    m2 = rng.random((8, 1, N, N), F32)
    out = kernel(V, m1, m2)
    print("kernel ran:", out.shape, out.dtype, float(np.abs(out).mean()))
